# revision 1
# baseline (speedup 1.0000x reference)
"""Cross multi-head attention on 8 Trainium2 NeuronCores.

Problem: y = CrossMHA(x, memory) with B=4, Tq=1024, Tk=2048, D=1024, H=16.

Sharding: 8 cores = (batch b in 0..3) x (head-half s in 0..1).  Each core
handles one batch element and 8 of the 16 heads: it computes the q/k/v
projections for its head columns, attention for its heads, and a partial
output projection y_partial = ctx @ W_o[rows of its heads].  The host sums
the two head-half partials per batch and adds b_o.

Per-core kernel (all matmul operands float32r -> full PE rate, fp32 PSUM):
  - Qt = W_q_s^T @ x^T          [512, 1024]  (q transposed: heads on partitions)
  - Kt = W_k_s^T @ mem^T        [512, 2048]
  - V  = mem @ W_v_s            [2048, 8, 65] (65th column = ones, so that the
                                A@V matmul also produces the softmax denom)
  - per (head, q-tile of 512):
      St[k,q]   = Kt_h^T-slices @ Qt_h     (keys on partitions, 16 k-chunks,
                                            two chunks share one PSUM tile)
      Ae        = exp(St/8 + mask_bias[k]) (ACT pass over the chunk pair; the
                                            mask bias is per-partition here)
      ctx/den   = V_aug^T @ Ae             (accumulated over the 16 k-chunks;
                                            row 64 is the softmax denominator)
      ctx_norm  = ctx * (1/den)            (1/den broadcast via tiny matmul)
  - y_partial = ctx_norm^T @ W_o_rows      [1024, 1024]

Schedule: the first attention group is interleaved with the last two K/V
projection k-tiles so the scalar engine (exp) starts early; the output
projection for the first q-half runs while the second q-half's attention
groups still execute.  Score matmuls of adjacent heads alternate between
the two 64-row halves of the PE array (tile_position row packing) so they
run concurrently on hardware.
"""

import sys

if "/opt/trn_rl_repo" not in sys.path:
    sys.path.insert(0, "/opt/trn_rl_repo")

import numpy as np

import concourse.bacc as bacc
import concourse.mybir as mybir
import concourse.tile as tile
from concourse.bass_utils import run_bass_kernel_spmd

N_CORES = 8
B = 4
TQ = 1024
TK = 2048
D = 1024
H = 16
HD = 64
H_LOC = 8          # heads per core
DH = H_LOC * HD    # 512: per-core head dims
F32 = mybir.dt.float32
F32R = mybir.dt.float32r
EXP = mybir.ActivationFunctionType.Exp

_PROGRAM_CACHE = {}


def _build_program(trivial_mask, hpg=4, ps_bufs=2, av_bufs=4, interleave=True, opr=None):
    nc = bacc.Bacc()
    OPR = F32R if opr is None else opr

    xT = nc.dram_tensor("xT", [D, TQ], OPR, kind="ExternalInput").ap()
    memT = nc.dram_tensor("memT", [D, TK], OPR, kind="ExternalInput").ap()
    wq_d = nc.dram_tensor("wq", [D, DH], OPR, kind="ExternalInput").ap()
    wk_d = nc.dram_tensor("wk", [D, DH], OPR, kind="ExternalInput").ap()
    wv_d = nc.dram_tensor("wv", [D, DH], OPR, kind="ExternalInput").ap()
    wo_d = nc.dram_tensor("wo", [DH, D], OPR, kind="ExternalInput").ap()
    bq_d = nc.dram_tensor("bq", [DH], F32, kind="ExternalInput").ap()
    bk_d = nc.dram_tensor("bk", [DH], F32, kind="ExternalInput").ap()
    bv_d = nc.dram_tensor("bv", [DH], F32, kind="ExternalInput").ap()
    maskb_d = nc.dram_tensor("maskb", [TK], F32, kind="ExternalInput").ap()
    ones_d = nc.dram_tensor("ones_in", [64], OPR, kind="ExternalInput").ap()
    y_d = nc.dram_tensor("y", [TQ, D], F32, kind="ExternalOutput").ap()

    with tile.TileContext(nc, pool_alloc_mode="queue") as tc, \
            nc.allow_low_precision(reason="float32r operands; fp32 PSUM accum"):
        # ---- constants / biases ------------------------------------------
        singles = tc.alloc_tile_pool(name="singles", bufs=1)
        bq_sb = singles.tile([128, 4], F32, tag="bq")
        nc.sync.dma_start(out=bq_sb, in_=bq_d.rearrange("(c p) -> p c", p=128))
        bk_sb = singles.tile([128, 4], F32, tag="bk")
        nc.sync.dma_start(out=bk_sb, in_=bk_d.rearrange("(c p) -> p c", p=128))
        maskb_sb = singles.tile([128, 16], F32, tag="maskb")
        nc.sync.dma_start(out=maskb_sb, in_=maskb_d.rearrange("(c p) -> p c", p=128))
        bv_bc = singles.tile([128, DH], F32, tag="bv")
        nc.gpsimd.dma_start(out=bv_bc, in_=bv_d.partition_broadcast(128))
        ones64 = singles.tile([1, 64], OPR, tag="ones64")
        nc.sync.dma_start(out=ones64, in_=ones_d.rearrange("(p n) -> p n", p=1))
        onescol = singles.tile([128, H_LOC, 1], OPR, tag="onescol")
        nc.sync.dma_start(
            out=onescol,
            in_=ones_d.partition_broadcast(128)[:, 0:H_LOC].rearrange(
                "p (n u) -> p n u", u=1
            ),
        )

        # ---- persistent activations --------------------------------------
        p_qt = tc.alloc_tile_pool(name="qt", bufs=1)
        Qt = [p_qt.tile([128, TQ], OPR, tag=f"qt{i}", name=f"qt{i}")
              for i in range(4)]
        p_kt = tc.alloc_tile_pool(name="kt", bufs=1)
        Kt = [p_kt.tile([128, TK], OPR, tag=f"kt{i}", name=f"kt{i}")
              for i in range(4)]
        p_v = tc.alloc_tile_pool(name="v", bufs=1)
        V = [p_v.tile([128, H_LOC, HD + 1], OPR, tag=f"v{i}", name=f"v{i}")
             for i in range(16)]
        p_ctx = tc.alloc_tile_pool(name="ctx", bufs=1)
        ctxT = [p_ctx.tile([128, TQ], OPR, tag=f"ctx{i}", name=f"ctx{i}")
                for i in range(4)]
        p_in = tc.alloc_tile_pool(name="inp", bufs=2)
        p_st = tc.alloc_tile_pool(name="st", bufs=4)
        p_cun = tc.alloc_tile_pool(name="cun", bufs=6)
        p_small = tc.alloc_tile_pool(name="small", bufs=2)

        # Two PSUM pools for the whole program: "ps" (2 x 2-bank slots) for
        # projections / score pairs / broadcasts, "av" (4 x 1-bank slots)
        # for the per-head attention accumulators.
        p_ps = tc.alloc_tile_pool(name="ps", bufs=ps_bufs, space="PSUM")
        p_av = tc.alloc_tile_pool(name="av", bufs=av_bufs, space="PSUM")

        def ps_tile(shape, name):
            return p_ps.tile(shape, F32, tag="ps", name=name,
                             padded_shape=[128, 1024])

        # ---- Q projection: Qt[dq, t] = sum_d W_q[d, dq] x^T[d, t] + b_q --
        p_wq = tc.alloc_tile_pool(name="wq", bufs=1)
        wq_r = wq_d.rearrange("(c p) n -> c p n", p=128)
        wq_sb = []
        for c in range(8):
            t = p_wq.tile([128, DH], OPR, tag=f"wq{c}", name=f"wq{c}")
            nc.sync.dma_start(out=t, in_=wq_r[c])
            wq_sb.append(t)

        for tt in range(2):
            xin = []
            for c in range(8):
                t = p_in.tile([128, 512], OPR, tag=f"in{c}", name=f"inx{c}")
                nc.sync.dma_start(
                    out=t, in_=xT[c * 128:(c + 1) * 128, tt * 512:(tt + 1) * 512]
                )
                xin.append(t)
            for dqc in range(4):
                ps = ps_tile([128, 512], f"ps_q{tt}{dqc}")
                for c in range(8):
                    nc.tensor.matmul(
                        ps,
                        lhsT=wq_sb[c][:, dqc * 128:(dqc + 1) * 128],
                        rhs=xin[c],
                        start=(c == 0),
                        stop=(c == 7),
                    )
                nc.vector.tensor_scalar_add(
                    out=Qt[dqc][:, tt * 512:(tt + 1) * 512],
                    in0=ps,
                    scalar1=bq_sb[:, dqc:dqc + 1],
                )
        p_wq.release()

        # ---- K/V projection (one k-tile of 512 keys) ----------------------
        p_wkv = tc.alloc_tile_pool(name="wkv", bufs=1)
        wk_r = wk_d.rearrange("(c p) n -> c p n", p=128)
        wv_r = wv_d.rearrange("(c p) n -> c p n", p=128)
        wk_sb, wv_sb = [], []
        for c in range(8):
            t = p_wkv.tile([128, DH], OPR, tag=f"wk{c}", name=f"wk{c}")
            nc.sync.dma_start(out=t, in_=wk_r[c])
            wk_sb.append(t)
            t = p_wkv.tile([128, DH], OPR, tag=f"wv{c}", name=f"wv{c}")
            nc.sync.dma_start(out=t, in_=wv_r[c])
            wv_sb.append(t)

        def kv_tile(kt):
            min_ = []
            for c in range(8):
                t = p_in.tile([128, 512], OPR, tag=f"in{c}", name=f"inm{c}")
                nc.sync.dma_start(
                    out=t, in_=memT[c * 128:(c + 1) * 128, kt * 512:(kt + 1) * 512]
                )
                min_.append(t)
            for dkc in range(4):
                ps = ps_tile([128, 512], f"ps_k{kt}{dkc}")
                for c in range(8):
                    nc.tensor.matmul(
                        ps,
                        lhsT=wk_sb[c][:, dkc * 128:(dkc + 1) * 128],
                        rhs=min_[c],
                        start=(c == 0),
                        stop=(c == 7),
                    )
                nc.vector.tensor_scalar_add(
                    out=Kt[dkc][:, kt * 512:(kt + 1) * 512],
                    in0=ps,
                    scalar1=bk_sb[:, dkc:dkc + 1],
                )
            for j in range(4):
                kk = kt * 4 + j
                ps = ps_tile([128, 512], f"ps_v{kk}")
                for c in range(8):
                    nc.tensor.matmul(
                        ps,
                        lhsT=min_[c][:, j * 128:(j + 1) * 128],
                        rhs=wv_sb[c],
                        start=(c == 0),
                        stop=(c == 7),
                    )
                vt = V[kk]
                nc.vector.tensor_add(
                    out=vt[:, :, 0:HD],
                    in0=ps.rearrange("p (h e) -> p h e", h=H_LOC),
                    in1=bv_bc.rearrange("p (h e) -> p h e", h=H_LOC),
                )
                nc.vector.tensor_copy(out=vt[:, :, HD:HD + 1], in_=onescol)

        # ---- attention helpers --------------------------------------------
        def att_alloc_avs(qt_i, hg):
            return {
                h: p_av.tile([65, 512], F32, tag="av", name=f"av{h}_{qt_i}")
                for h in [hg * hpg + i for i in range(hpg)]
            }

        def att_pairs(qt_i, hg, avs, pps):
            qsl = slice(qt_i * 512, (qt_i + 1) * 512)
            for pp in pps:
                kks = (2 * pp, 2 * pp + 1)
                for h in [hg * hpg + i for i in range(hpg)]:
                    ht, hb = h // 2, (h % 2) * 64
                    ps = ps_tile([128, 1024], f"sc{h}_{pp}_{qt_i}")
                    for half, kk in enumerate(kks):
                        nc.tensor.matmul(
                            ps[:, half * 512:(half + 1) * 512],
                            lhsT=Kt[ht][hb:hb + 64, kk * 128:(kk + 1) * 128],
                            rhs=Qt[ht][hb:hb + 64, qsl],
                            start=True,
                            stop=True,
                            tile_position=(hb, 0),
                        )
                    st = p_st.tile([128, 1024], OPR, tag="st",
                                   name=f"st{h}_{pp}")
                    if trivial_mask:
                        nc.scalar.activation(
                            out=st, in_=ps, func=EXP, bias=0.0, scale=0.125
                        )
                    else:
                        for half, kk in enumerate(kks):
                            nc.scalar.activation(
                                out=st[:, half * 512:(half + 1) * 512],
                                in_=ps[:, half * 512:(half + 1) * 512],
                                func=EXP,
                                bias=maskb_sb[:, kk:kk + 1],
                                scale=0.125,
                            )
                    for half, kk in enumerate(kks):
                        nc.tensor.matmul(
                            avs[h],
                            lhsT=V[kk][:, h, :],
                            rhs=st[:, half * 512:(half + 1) * 512],
                            start=(kk == 0),
                            stop=(kk == 15),
                        )

        def att_norm(qt_i, avs):
            qsl = slice(qt_i * 512, (qt_i + 1) * 512)
            cuns = {}
            for h, av in avs.items():
                cun = p_cun.tile([65, 512], F32, tag="cun",
                                 name=f"cun{h}_{qt_i}")
                nc.vector.tensor_copy(out=cun, in_=av)
                cuns[h] = cun
            for h, cun in cuns.items():
                ht, hb = h // 2, (h % 2) * 64
                recip = p_small.tile([1, 512], OPR, tag="recip",
                                     name=f"recip{h}")
                nc.vector.reciprocal(out=recip, in_=cun[64:65, :])
                rb_ps = p_av.tile([64, 512], F32, tag="av", name=f"rb_ps{h}")
                nc.tensor.matmul(rb_ps, lhsT=ones64, rhs=recip,
                                 start=True, stop=True)
                rb = p_small.tile([64, 512], F32, tag="rb", name=f"rb{h}")
                nc.vector.tensor_copy(out=rb, in_=rb_ps)
                nc.vector.tensor_mul(
                    out=ctxT[ht][hb:hb + 64, qsl], in0=cun[0:64, :], in1=rb
                )

        def out_proj(p_y, wo_sb, qcs):
            for qc in qcs:
                ysb = p_y.tile([128, D], F32, tag="y", name=f"y{qc}")
                for ot in range(2):
                    ps = ps_tile([128, 512], f"ps_o{qc}{ot}")
                    for c in range(4):
                        nc.tensor.matmul(
                            ps,
                            lhsT=ctxT[c][:, qc * 128:(qc + 1) * 128],
                            rhs=wo_sb[c][:, ot * 512:(ot + 1) * 512],
                            start=(c == 0),
                            stop=(c == 3),
                        )
                    nc.vector.tensor_copy(
                        out=ysb[:, ot * 512:(ot + 1) * 512], in_=ps
                    )
                nc.sync.dma_start(out=y_d[qc * 128:(qc + 1) * 128, :], in_=ysb)

        # ---- schedule -----------------------------------------------------
        n_groups = H_LOC // hpg
        kv_tile(0)
        kv_tile(1)

        g0 = att_alloc_avs(0, 0)
        if interleave:
            att_pairs(0, 0, g0, range(0, 4))  # kk 0..7 need only k-tiles 0,1

        kv_tile(2)
        kv_tile(3)
        p_wkv.release()

        p_wo = tc.alloc_tile_pool(name="wo", bufs=1)
        wo_r = wo_d.rearrange("(c p) n -> c p n", p=128)
        wo_sb = []
        for c in range(4):
            t = p_wo.tile([128, D], OPR, tag=f"wo{c}", name=f"wo{c}")
            nc.sync.dma_start(out=t, in_=wo_r[c])
            wo_sb.append(t)
        p_y = tc.alloc_tile_pool(name="y", bufs=3)

        att_pairs(0, 0, g0, range(4, 8) if interleave else range(8))
        att_norm(0, g0)
        for hg in range(1, n_groups):
            g = att_alloc_avs(0, hg)
            att_pairs(0, hg, g, range(8))
            att_norm(0, g)

        out_proj(p_y, wo_sb, range(0, 4))    # q rows 0..511 (qt 0)

        for hg in range(n_groups):
            g = att_alloc_avs(1, hg)
            att_pairs(1, hg, g, range(8))
            att_norm(1, g)

        out_proj(p_y, wo_sb, range(4, 8))    # q rows 512..1023 (qt 1)

        for pool in (p_y, p_wo, p_av, p_ps, p_small, p_cun, p_st, p_in,
                     p_ctx, p_v, p_kt, p_qt, singles):
            pool.release()

    nc.compile()
    return nc


BUILD_OPTS = dict(hpg=2, ps_bufs=3, av_bufs=2, interleave=True)


def get_program(trivial_mask=True):
    key = ("nc", bool(trivial_mask), tuple(sorted(BUILD_OPTS.items())))
    if key not in _PROGRAM_CACHE:
        _PROGRAM_CACHE[key] = _build_program(trivial_mask, **BUILD_OPTS)
    return _PROGRAM_CACHE[key]


def make_in_maps(x, memory, memory_padding_mask, W_q, b_q, W_kv, b_kv, W_o):
    x = np.asarray(x, dtype=np.float32)
    memory = np.asarray(memory, dtype=np.float32)
    mask = np.asarray(memory_padding_mask)
    W_q = np.asarray(W_q, dtype=np.float32)
    b_q = np.asarray(b_q, dtype=np.float32)
    W_kv = np.asarray(W_kv, dtype=np.float32)
    b_kv = np.asarray(b_kv, dtype=np.float32)
    W_o = np.asarray(W_o, dtype=np.float32)

    in_maps = []
    for c in range(N_CORES):
        b, s = c // 2, c % 2
        sl = slice(s * DH, (s + 1) * DH)
        vsl = slice(D + s * DH, D + (s + 1) * DH)
        in_maps.append({
            "xT": np.ascontiguousarray(x[b].T),
            "memT": np.ascontiguousarray(memory[b].T),
            "wq": np.ascontiguousarray(W_q[:, sl]),
            "wk": np.ascontiguousarray(W_kv[:, sl]),
            "wv": np.ascontiguousarray(W_kv[:, vsl]),
            "wo": np.ascontiguousarray(W_o[sl, :]),
            "bq": np.ascontiguousarray(b_q[sl]),
            "bk": np.ascontiguousarray(b_kv[sl]),
            "bv": np.ascontiguousarray(b_kv[vsl]),
            "maskb": np.where(mask[b], 0.0, -30000.0).astype(np.float32),
            "ones_in": np.ones(64, dtype=np.float32),
        })
    return in_maps


def kernel(x, memory, memory_padding_mask, W_q, b_q, W_kv, b_kv, W_o, b_o):
    trivial_mask = bool(np.asarray(memory_padding_mask).all())
    nc = get_program(trivial_mask)
    in_maps = make_in_maps(
        x, memory, memory_padding_mask, W_q, b_q, W_kv, b_kv, W_o
    )
    res = run_bass_kernel_spmd(nc, in_maps, list(range(N_CORES)))
    ys = [res.results[c]["y"] for c in range(N_CORES)]
    b_o = np.asarray(b_o, dtype=np.float32)
    out = np.stack([ys[2 * b] + ys[2 * b + 1] for b in range(B)])
    out += b_o[None, None, :]
    return out.astype(np.float32)



# revision 23
# speedup vs baseline: 1.3134x; 1.3134x over previous
"""Cross multi-head attention on 8 Trainium2 NeuronCores.

Problem: y = CrossMHA(x, memory) with B=4, Tq=1024, Tk=2048, D=1024, H=16.

Sharding: 8 cores = (batch b in 0..3) x (head-half s in 0..1).  Each core
handles one batch element and 8 of the 16 heads.  The host sums the two
head-half partials per batch and adds b_o.

Per-core kernel (mixed precision, tuned for the TRN2 cost model):
  - Projections run as fp8(e4m3) hi/lo 3-product matmuls in DoubleRow perf
    mode (x = x_hi + x_lo, W*64 = W_hi + W_lo split on the host; the
    dropped lo*lo term is ~1e-3 relative).  DoubleRow packs two 128-row
    contraction chunks per instruction at 0.5 cycles/row.
  - Q^T/K^T live in SBUF as bf16 [dh, T] tiles (two heads per tile);
    score matmuls are bf16 with 512-wide outputs (full PE rate).
  - exp runs on the scalar engine from the score PSUM, scale=1/8 (bias
    carries the padding mask when nontrivial), writing bf16 st tiles.
  - A@V runs queries-on-partitions: out [128q, 65] bf16 matmuls (65th V
    column is ones so row 64 accumulates the softmax denominator), making
    the denominator a per-partition scalar: reciprocal + one fused
    multiply-cast per tile.
  - ctx [q, dh] is transposed back to [dh, q] with PE transpose-mode
    matmuls against a bf16 identity; the output projection runs in bf16
    with 1024-wide outputs; y streams out per 128-row block.

Schedule: scores/exp run pp-major in waves of four heads so the scalar
engine (the 133us exp floor) starts ~13us in and stays saturated while
the K/V projections and DMAs stream underneath; A@V, normalize,
transpose, and the output projection trail each wave.
"""

import sys

if "/opt/trn_rl_repo" not in sys.path:
    sys.path.insert(0, "/opt/trn_rl_repo")

import numpy as np
import ml_dtypes

import concourse.bacc as bacc
import concourse.mybir as mybir
import concourse.tile as tile
from concourse.bass_utils import run_bass_kernel_spmd

N_CORES = 8
B = 4
TQ = 1024
TK = 2048
D = 1024
H = 16
HD = 64
H_LOC = 8          # heads per core
DH = H_LOC * HD    # 512: per-core head dims
WS = 64.0          # host-side weight scale (fp8 subnormal avoidance)
F32 = mybir.dt.float32
F32R = mybir.dt.float32r
BF16 = mybir.dt.bfloat16
F8 = mybir.dt.float8e4
EXP = mybir.ActivationFunctionType.Exp
DRow = mybir.MatmulPerfMode.DoubleRow
ALU = mybir.AluOpType
f8np = ml_dtypes.float8_e4m3
bf16np = ml_dtypes.bfloat16

_PROGRAM_CACHE = {}


def _build_program(trivial_mask, sc_bufs=2, pp_bufs=2, avtr_bufs=2, st_bufs=5):
    nc = bacc.Bacc()

    xh_d = nc.dram_tensor("xh", [128, 4, 2, TQ], F8, kind="ExternalInput").ap()
    xl_d = nc.dram_tensor("xl", [128, 4, 2, TQ], F8, kind="ExternalInput").ap()
    mh_d = nc.dram_tensor("mh", [128, 4, 2, TK], F8, kind="ExternalInput").ap()
    ml_d = nc.dram_tensor("ml", [128, 4, 2, TK], F8, kind="ExternalInput").ap()
    w_d = {}
    for w in ("wq", "wk", "wv"):
        for p in ("h", "l"):
            w_d[w + p] = nc.dram_tensor(
                w + p, [128, 4, 2, DH], F8, kind="ExternalInput"
            ).ap()
    wo_d = nc.dram_tensor("wo", [128, 4, D], BF16, kind="ExternalInput").ap()
    bq_d = nc.dram_tensor("bq", [128, 4], F32, kind="ExternalInput").ap()
    bk_d = nc.dram_tensor("bk", [128, 4], F32, kind="ExternalInput").ap()
    bv_d = nc.dram_tensor("bv", [DH], F32, kind="ExternalInput").ap()
    id_d = nc.dram_tensor("ident", [128, 128], BF16, kind="ExternalInput").ap()
    maskb_d = nc.dram_tensor("maskb", [TK], F32, kind="ExternalInput").ap()
    y_d = nc.dram_tensor("y", [TQ, D], F32, kind="ExternalOutput").ap()

    with tile.TileContext(nc, pool_alloc_mode="queue") as tc, \
            nc.allow_low_precision(reason="fp8/bf16 operands; fp32 PSUM accum"):
        # ---- pools (persistent first; weight/input pools on top of the
        # stack so they release after the projections) ---------------------
        singles = tc.alloc_tile_pool(name="singles", bufs=1)
        p_qt = tc.alloc_tile_pool(name="qt", bufs=1)
        p_kt = tc.alloc_tile_pool(name="kt", bufs=1)
        p_v = tc.alloc_tile_pool(name="v", bufs=1)
        p_ctx = tc.alloc_tile_pool(name="ctx", bufs=1)
        p_st1 = tc.alloc_tile_pool(name="st", bufs=st_bufs)
        pools = {}
        st_policy = {}
        p_cp = tc.alloc_tile_pool(name="cp", bufs=2)
        p_small = tc.alloc_tile_pool(name="small", bufs=4)
        p_sc = tc.alloc_tile_pool(name="sc", bufs=sc_bufs, space="PSUM")
        p_pp = tc.alloc_tile_pool(name="pp", bufs=pp_bufs, space="PSUM")
        p_avtr = tc.alloc_tile_pool(name="avtr", bufs=avtr_bufs, space="PSUM")
        p_wv = tc.alloc_tile_pool(name="wv", bufs=1)
        p_wk = tc.alloc_tile_pool(name="wk", bufs=1)
        p_wq = tc.alloc_tile_pool(name="wq", bufs=1)
        p_x = tc.alloc_tile_pool(name="x", bufs=1)
        w_pools = {"wv": p_wv, "wk": p_wk, "wq": p_wq}

        # ---- input DMAs in dependency-priority order ---------------------
        w_sb = {}

        def load_w(name):
            t = w_pools[name[:2]].tile([128, 4, 2, DH], F8, tag=name,
                                       name=name)
            nc.sync.dma_start(out=t, in_=w_d[name])
            w_sb[name] = t

        load_w("wqh")
        load_w("wql")
        xh_sb = p_x.tile([128, 4, 2, TQ], F8, tag="xh", name="xh")
        nc.sync.dma_start(out=xh_sb, in_=xh_d)
        xl_sb = p_x.tile([128, 4, 2, TQ], F8, tag="xl", name="xl")
        nc.sync.dma_start(out=xl_sb, in_=xl_d)
        load_w("wkh")
        load_w("wkl")
        mh_sb = p_x.tile([128, 4, 2, TK], F8, tag="mh", name="mh")
        ml_sb = p_x.tile([128, 4, 2, TK], F8, tag="ml", name="ml")

        def load_m(kt):
            ksl = slice(kt * 512, (kt + 1) * 512)
            nc.sync.dma_start(out=mh_sb[:, :, :, ksl], in_=mh_d[:, :, :, ksl])
            nc.sync.dma_start(out=ml_sb[:, :, :, ksl], in_=ml_d[:, :, :, ksl])

        load_m(0)
        bq_sb = singles.tile([128, 4], F32, tag="bq")
        nc.sync.dma_start(out=bq_sb, in_=bq_d)
        bk_sb = singles.tile([128, 4], F32, tag="bk")
        nc.sync.dma_start(out=bk_sb, in_=bk_d)
        maskb_sb = singles.tile([128, 16], F32, tag="maskb")
        if not trivial_mask:
            nc.sync.dma_start(
                out=maskb_sb, in_=maskb_d.rearrange("(c p) -> p c", p=128))

        def sc_ps(name):
            return p_sc.tile([128, 1024], F32, tag="sc", name=name,
                             padded_shape=[128, 1024])

        def pp_ps(name):
            return p_pp.tile([128, 512], F32, tag="pp", name=name,
                             padded_shape=[128, 512])

        # ---- projections -------------------------------------------------
        def proj3(ps, wname, x_hi, x_lo, j, nsl):
            """psum += 64*(x @ W)[j-block, nsl] via 3-product fp8 DoubleRow."""
            wh, wl = w_sb[wname + "h"], w_sb[wname + "l"]
            jsl = slice(j * 128, (j + 1) * 128)
            first = True
            for c in range(4):
                for wt, xt in ((wh, x_hi), (wl, x_hi), (wh, x_lo)):
                    nc.tensor.matmul(
                        ps,
                        lhsT=wt[:, c, :, jsl],
                        rhs=xt[:, c, :, nsl],
                        start=first,
                        stop=(c == 3 and wt is wh and xt is x_lo),
                        perf_mode=DRow,
                    )
                    first = False

        Qt = [None] * 4
        Kt = [None] * 4
        V = [None] * 16
        ctxT = [None] * 4

        def q_tile(tt, j):
            if Qt[j] is None:
                Qt[j] = p_qt.tile([128, TQ], BF16, tag=f"qt{j}", name=f"qt{j}")
            nsl = slice(tt * 512, (tt + 1) * 512)
            ps = pp_ps(f"ps_q{tt}{j}")
            proj3(ps, "wq", xh_sb, xl_sb, j, nsl)
            nc.vector.tensor_scalar(
                out=Qt[j][:, nsl], in0=ps, scalar1=1.0 / WS,
                scalar2=bq_sb[:, j:j + 1], op0=ALU.mult, op1=ALU.add,
            )

        def k_tile(kt, j):
            if Kt[j] is None:
                Kt[j] = p_kt.tile([128, TK], BF16, tag=f"kt{j}", name=f"kt{j}")
            nsl = slice(kt * 512, (kt + 1) * 512)
            ps = pp_ps(f"ps_k{kt}{j}")
            proj3(ps, "wk", mh_sb, ml_sb, j, nsl)
            nc.vector.tensor_scalar(
                out=Kt[j][:, nsl], in0=ps, scalar1=1.0 / WS,
                scalar2=bk_sb[:, j:j + 1], op0=ALU.mult, op1=ALU.add,
            )

        def v_tile(kk):
            # out [keys 128, hd 512]: lhsT = mem chunk, rhs = wv
            V[kk] = p_v.tile([128, H_LOC, HD + 1], BF16, tag=f"v{kk}",
                             name=f"v{kk}")
            ksl = slice(kk * 128, (kk + 1) * 128)
            ps = pp_ps(f"ps_v{kk}")
            first = True
            for c in range(4):
                for wn, xt in (("wvh", mh_sb), ("wvl", mh_sb), ("wvh", ml_sb)):
                    nc.tensor.matmul(
                        ps,
                        lhsT=xt[:, c, :, ksl],
                        rhs=w_sb[wn][:, c],
                        start=first,
                        stop=(c == 3 and wn == "wvh" and xt is ml_sb),
                        perf_mode=DRow,
                    )
                    first = False
            vt = V[kk]
            nc.vector.scalar_tensor_tensor(
                out=vt[:, :, 0:HD],
                in0=ps.rearrange("p (h e) -> p h e", h=H_LOC),
                scalar=1.0 / WS,
                in1=bv_bc.rearrange("p (h e) -> p h e", h=H_LOC),
                op0=ALU.mult, op1=ALU.add,
            )
            nc.vector.memset(vt[:, :, HD:HD + 1], 1.0)

        # ---- attention ---------------------------------------------------
        st_tiles = {}

        def scores_pp(h, qt, pp):
            """one kk pair of score matmuls + exp for head h."""
            key = (h, qt)
            if key not in st_tiles:
                st_tiles[key] = p_st1.tile([128, 16, 512], BF16, tag="st",
                                           name=f"st{h}_{qt}")
            st_sb = st_tiles[key]
            ht, hb = h // 2, (h % 2) * 64
            qsl = slice(qt * 512, (qt + 1) * 512)
            ps = sc_ps(f"sc{h}_{pp}_{qt}")
            for half in range(2):
                kk = 2 * pp + half
                nc.tensor.matmul(
                    ps[:, half * 512:(half + 1) * 512],
                    lhsT=Kt[ht][hb:hb + 64, kk * 128:(kk + 1) * 128],
                    rhs=Qt[ht][hb:hb + 64, qsl],
                    start=True,
                    stop=True,
                )
            if trivial_mask:
                nc.scalar.activation(
                    out=st_sb[:, 2 * pp:2 * pp + 2, :].rearrange(
                        "p a b -> p (a b)"),
                    in_=ps, func=EXP, bias=0.0, scale=0.125,
                )
            else:
                for half in range(2):
                    kk = 2 * pp + half
                    nc.scalar.activation(
                        out=st_sb[:, kk, :],
                        in_=ps[:, half * 512:(half + 1) * 512],
                        func=EXP, bias=maskb_sb[:, kk:kk + 1], scale=0.125,
                    )

        cp_tiles = {}

        def avq(h, qt, qc):
            """A@V for one 128-query block of head h, normalize into cp."""
            hp = h // 2
            if (hp, qt) not in cp_tiles:
                cp_tiles[(hp, qt)] = p_cp.tile(
                    [128, 4, 128], BF16, tag="cp", name=f"cp{hp}_{qt}")
            cp = cp_tiles[(hp, qt)]
            st_sb = st_tiles[(h, qt)]
            hb2 = (h % 2) * 64
            av = p_avtr.tile([128, HD + 1], F32, tag="av",
                             name=f"av{h}_{qt}_{qc}",
                             padded_shape=[128, 512])
            for kk in range(16):
                nc.tensor.matmul(
                    av,
                    lhsT=st_sb[:, kk, qc * 128:(qc + 1) * 128],
                    rhs=V[kk][:, h, :],
                    start=(kk == 0),
                    stop=(kk == 15),
                )
            recip = p_small.tile([128, 1], F32, tag="recip",
                                 name=f"rc{h}_{qt}_{qc}")
            nc.vector.reciprocal(out=recip, in_=av[:, HD:HD + 1])
            nc.vector.tensor_scalar(
                out=cp[:, qc, hb2:hb2 + 64], in0=av[:, 0:HD],
                scalar1=recip, scalar2=None, op0=ALU.mult,
            )
            if qc == 3:
                st_tiles.pop((h, qt))

        def trq(hp, qt, qc):
            cp = cp_tiles[(hp, qt)]
            if ctxT[hp] is None:
                ctxT[hp] = p_ctx.tile([128, TQ], BF16, tag=f"ctx{hp}",
                                      name=f"ctx{hp}")
            tr = p_avtr.tile([128, 128], BF16, tag="av",
                             name=f"tr{hp}_{qt}_{qc}",
                             padded_shape=[128, 1024])
            nc.tensor.matmul(tr, lhsT=cp[:, qc, :], rhs=id_sb,
                             start=True, stop=True, is_transpose=True)
            nc.vector.tensor_copy(
                out=ctxT[hp][:, qt * 512 + qc * 128:
                             qt * 512 + (qc + 1) * 128],
                in_=tr,
            )
            if qc == 3:
                cp_tiles.pop((hp, qt))

        def op_half(qcg, ot):
            qsl = slice(qcg * 128, (qcg + 1) * 128)
            ysb = p_y.tile([128, 512], F32, tag="y", name=f"y{qcg}_{ot}")
            ps = pp_ps(f"ps_o{qcg}_{ot}")
            osl = slice(ot * 512, (ot + 1) * 512)
            for c in range(4):
                nc.tensor.matmul(
                    ps, lhsT=ctxT[c][:, qsl], rhs=wo_sb[:, c, osl],
                    start=(c == 0), stop=(c == 3),
                )
            nc.vector.tensor_copy(out=ysb, in_=ps)
            nc.sync.dma_start(out=y_d[qsl, osl], in_=ysb)

        # ---- schedule ----------------------------------------------------
        # Staggered-entry row template: head slots enter one per row-group so
        # the previous wave's AV units interleave 1:1 with real scores
        # (keeps the scalar engine at exp cadence across wave boundaries).
        ROWS = [(0, 0), (0, 1), (0, 2), (0, 3),
                (1, 0), (1, 1), (0, 4), (0, 5),
                (2, 0), (2, 1), (1, 2), (0, 6),
                (3, 0), (2, 2), (1, 3), (0, 7),
                (3, 1), (2, 3), (1, 4), (1, 5),
                (3, 2), (3, 3), (2, 4), (2, 5),
                (1, 6), (1, 7), (2, 6), (3, 4),
                (2, 7), (3, 5), (3, 6), (3, 7)]

        def wave(heads, qt, pres, units):
            """pres: per-entry-slot (1..3) unit lists, emitted during the
            preceding row group; units: spread 1-per-score afterwards."""
            queue = list(pres.get(1, ())) + list(pres.get(2, ())) \
                + list(pres.get(3, ()))
            queue += list(units)
            for k, (hi, pp) in enumerate(ROWS):
                scores_pp(heads[hi], qt, pp)
                if k < len(queue):
                    queue[k]()
            for u in queue[len(ROWS):]:
                u()

        def wave_flat(heads, qt, units):
            units = list(units)
            for pp in range(8):
                for h in heads:
                    scores_pp(h, qt, pp)
                    if units:
                        units.pop(0)()
            for u in units:
                u()

        for j in range(4):
            q_tile(0, j)
        for j in range(4):
            k_tile(0, j)
        load_w("wvh")
        load_w("wvl")
        bv_bc = singles.tile([128, DH], F32, tag="bv")
        nc.gpsimd.dma_start(out=bv_bc, in_=bv_d.partition_broadcast(128))
        id_sb = singles.tile([128, 128], BF16, tag="ident")
        nc.sync.dma_start(out=id_sb, in_=id_d)

        U = lambda f, *a: (lambda: f(*a))

        def avu(h, qt):
            return [U(avq, h, qt, qc) for qc in range(4)]

        def tru(hp, qt):
            return [U(trq, hp, qt, qc) for qc in range(4)]

        # wave A: heads 0..3 qt0; stream the remaining projections (+ their
        # DMAs) one tile per score so the proj psum pool never backs up.
        unitsA = [U(load_m, 1), U(v_tile, 0), U(v_tile, 1), U(v_tile, 2)]
        unitsA += [U(k_tile, 1, j) for j in range(4)]
        unitsA += [U(load_m, 2), U(v_tile, 3), U(v_tile, 4), U(v_tile, 5)]
        unitsA += [U(k_tile, 2, j) for j in range(4)]
        unitsA += [U(load_m, 3), U(v_tile, 6), U(v_tile, 7), U(v_tile, 8)]
        unitsA += [U(k_tile, 3, j) for j in range(4)]
        unitsA += [U(v_tile, kk) for kk in range(9, 16)]
        unitsA += [U(q_tile, 1, j) for j in range(4)]
        wave_flat((0, 1, 2, 3), 0, unitsA)
        p_x.release()
        p_wq.release()
        p_y = tc.alloc_tile_pool(name="y", bufs=4)
        wo_sb = p_y.tile([128, 4, D], BF16, tag="wo", name="wo")
        nc.sync.dma_start(out=wo_sb, in_=wo_d)

        # waves B..D: previous heads' AV/normalize/transpose (and the output
        # projection) ride the unit slots, ordered so st pool slots free just
        # in time for each entering head.
        wave((4, 5, 6, 7), 0,
             {1: avu(0, 0), 2: avu(1, 0), 3: avu(2, 0)},
             avu(3, 0) + tru(0, 0) + tru(1, 0))

        wave((0, 1, 2, 3), 1,
             {1: avu(4, 0), 2: avu(5, 0), 3: avu(6, 0)},
             avu(7, 0) + tru(2, 0) + tru(3, 0)
             + [U(op_half, qcg, ot) for qcg in range(4) for ot in range(2)])

        wave((6, 7, 4, 5), 1,
             {1: avu(0, 1), 2: avu(1, 1), 3: avu(2, 1)},
             avu(3, 1) + tru(0, 1) + tru(1, 1) + avu(6, 1))

        for u in (avu(7, 1) + tru(3, 1) + avu(4, 1) + avu(5, 1)
                  + tru(2, 1)
                  + [U(op_half, qcg, ot) for qcg in range(4, 8)
                     for ot in range(2)]):
            u()

        for pool in (p_y, p_wk, p_wv, p_avtr, p_pp, p_sc,
                     p_small, p_cp, p_st1, p_ctx, p_v, p_kt, p_qt, singles):
            pool.release()

    nc.compile()
    return nc


BUILD_OPTS = dict(sc_bufs=2, pp_bufs=2, avtr_bufs=2, st_bufs=5)


def get_program(trivial_mask=True):
    key = ("nc", bool(trivial_mask), tuple(sorted(BUILD_OPTS.items())))
    if key not in _PROGRAM_CACHE:
        _PROGRAM_CACHE[key] = _build_program(trivial_mask, **BUILD_OPTS)
    return _PROGRAM_CACHE[key]


def _hl(a):
    hi = a.astype(f8np)
    lo = (a - hi.astype(np.float32)).astype(f8np)
    return hi, lo


def _arrange_dr(a, tcols):
    # [1024 rows, N] -> [128, 4, 2, N]: row = 256c + 128g + p
    return np.ascontiguousarray(
        a.reshape(4, 2, 128, tcols).transpose(2, 0, 1, 3)
    )


def make_in_maps(x, memory, memory_padding_mask, W_q, b_q, W_kv, b_kv, W_o):
    x = np.asarray(x, dtype=np.float32)
    memory = np.asarray(memory, dtype=np.float32)
    mask = np.asarray(memory_padding_mask)
    W_q = np.asarray(W_q, dtype=np.float32)
    b_q = np.asarray(b_q, dtype=np.float32)
    W_kv = np.asarray(W_kv, dtype=np.float32)
    b_kv = np.asarray(b_kv, dtype=np.float32)
    W_o = np.asarray(W_o, dtype=np.float32)
    ident = np.ascontiguousarray(np.eye(128, dtype=np.float32)).astype(bf16np)

    in_maps = []
    for c in range(N_CORES):
        b, s = c // 2, c % 2
        sl = slice(s * DH, (s + 1) * DH)
        vsl = slice(D + s * DH, D + (s + 1) * DH)
        xT = np.ascontiguousarray(x[b].T)
        mT = np.ascontiguousarray(memory[b].T)
        xhi, xlo = _hl(xT)
        mhi, mlo = _hl(mT)
        wq_h, wq_l = _hl(W_q[:, sl] * WS)
        wk_h, wk_l = _hl(W_kv[:, sl] * WS)
        wv_h, wv_l = _hl(W_kv[:, vsl] * WS)
        in_maps.append({
            "xh": _arrange_dr(xhi, TQ), "xl": _arrange_dr(xlo, TQ),
            "mh": _arrange_dr(mhi, TK), "ml": _arrange_dr(mlo, TK),
            "wqh": _arrange_dr(wq_h, DH), "wql": _arrange_dr(wq_l, DH),
            "wkh": _arrange_dr(wk_h, DH), "wkl": _arrange_dr(wk_l, DH),
            "wvh": _arrange_dr(wv_h, DH), "wvl": _arrange_dr(wv_l, DH),
            "wo": np.ascontiguousarray(
                W_o[sl, :].reshape(4, 128, D).transpose(1, 0, 2)
            ).astype(bf16np),
            "bq": np.ascontiguousarray(b_q[sl].reshape(4, 128).T),
            "bk": np.ascontiguousarray(b_kv[:D][sl].reshape(4, 128).T),
            "bv": np.ascontiguousarray(b_kv[vsl]),
            "ident": ident,
            "maskb": np.where(mask[b], 0.0, -30000.0).astype(np.float32),
        })
    return in_maps


def kernel(x, memory, memory_padding_mask, W_q, b_q, W_kv, b_kv, W_o, b_o):
    trivial_mask = bool(np.asarray(memory_padding_mask).all())
    nc = get_program(trivial_mask)
    in_maps = make_in_maps(
        x, memory, memory_padding_mask, W_q, b_q, W_kv, b_kv, W_o
    )
    res = run_bass_kernel_spmd(nc, in_maps, list(range(N_CORES)))
    ys = [res.results[c]["y"] for c in range(N_CORES)]
    b_o = np.asarray(b_o, dtype=np.float32)
    out = np.stack([ys[2 * b] + ys[2 * b + 1] for b in range(B)])
    out += b_o[None, None, :]
    return out.astype(np.float32)
